# revision 18
# baseline (speedup 1.0000x reference)
"""Trainium2 Bass kernel for nn_AttentionBlock (sparse_attention).

Reference computation per batch b (channels-first x[b]: [C=512, T=4096]):
    xt = x[b].T                                  # [T, C]
    q = xt @ Wq.T + bq ; k = xt @ Wk.T + bk      # [T, 512]
    v = xt @ Wv.T + bv                           # [T, 512]
    S = q @ k.T / sqrt(512), causal (j <= i)     # [T, T]
    P = softmax(S, axis=QUERY i)  (per-column normalization)
    act = P @ v                                  # [T, 512]
    out[b] = concat(x[b], act.T, axis=0)         # [1024, T]

Sharding: pure data-parallel over batch B=8 across the 8 NeuronCores
(one batch per core, no collectives).

v3 per-core algorithm — all matmuls fp8e4 DoubleRow (f32 PSUM):
  1. QKV projections from x8 (host-cast fp8).  Contraction over C=512
     as 2 DoubleRow pairs.  Q^T,K^T stored fp8 paired over head-dim
     chunks; V rows fp16.  Q/K eviction on DVE: (psum/c4) + b/c4 ->
     fp8 (c4 = 512**0.25 splits the score scale between q and k).
     Weights+biases+mask ride in 2 packed DRAM params (DMA triggers
     cost ~650ns each on the Sync engine; fewer, bigger transfers).
  2. Score strips ST[j,i] = K^T.T @ Q^T (j-chunk of 128 keys, i from
     the diagonal to T in 1024-col PSUM chunks; 512-col DoubleRow MMs,
     h-outer so LDWEIGHTS amortizes 2 MMs).  Column softmax over i:
     additive causal mask on the diagonal block, exp(s-4) on ScalarE
     with accum_out producing Z_j partials (1024-col chunks halve the
     per-instruction ACTIVATE/READ_ACCUMULATOR overhead).  P~ stays in
     SBUF as fp8 pair tiles (triangle = 72KB/partition).
  3. V rows scaled by 1/Z_j on DVE, clipped to +-240 (fp8e4 overflows
     to inf/NaN, no saturation), cast fp8 into paired v8 tiles.
  4. act^T[v,i] = sum_j V'[j,v] * P~[j,i]: PSUM-accumulated DoubleRow
     matmuls from SBUF, interleaved between score-strip chunks so
     TensorE never stalls on the ScalarE exp chain.  DVE eviction.
  5. out rows 0..511 are a DRAM->DRAM copy of x[b] overlapping the
     whole kernel.
"""

import math

import numpy as np

import concourse.bass as bass
import concourse.mybir as mybir
from concourse import bacc, tile
from concourse.bass_utils import run_bass_kernel_spmd

P = 128
C = 512
T = 4096
KDIM = 512
VDIM = 512
NTC = T // P      # 32 time chunks of 128
NIB = T // 512    # 8 i-blocks of 512
F16 = mybir.dt.float16
F32 = mybir.dt.float32
F8 = mybir.dt.float8e4
EXP_SHIFT = -4.0  # constant logit shift: softmax-invariant, keeps exp in range
MASK_NEG = -10000.0
C4 = float(C) ** 0.25
FP8MAX = 240.0
DR = mybir.MatmulPerfMode.DoubleRow
WPK = 3 * KDIM    # packed weight row: wq | wk | wv
BPK = 4 + 4 + VDIM + P  # packed per-partition consts: bq | bk | bv | mask

_CACHE = {}


def _ts(i, size):
    return slice(i * size, (i + 1) * size)


def build_nc():
    nc = bacc.Bacc(
        "TRN2",
        target_bir_lowering=False,
        debug=False,
        num_devices=8,
    )

    x8_d = nc.declare_dram_parameter("x8", [C, T], F8, isOutput=False)
    x32_d = nc.declare_dram_parameter("x32", [C, T], F32, isOutput=False)
    w8_d = nc.declare_dram_parameter("w8", [C, WPK], F8, isOutput=False)
    bp_d = nc.declare_dram_parameter("bp", [P, BPK], F32, isOutput=False)
    out_d = nc.declare_dram_parameter("out", [C + VDIM, T], F32, isOutput=True)

    def re2(ap):
        return ap.rearrange("p (u n) -> p u n", u=2)

    with tile.TileContext(nc) as tc:
        from contextlib import ExitStack

        with ExitStack() as ctx:
            singles = ctx.enter_context(tc.tile_pool(name="singles", bufs=1))

            def single(shape, dtype, tag):
                return singles.tile(shape, dtype, name=tag, tag=tag)

            # paired fp8 layouts: plane u of tile h holds 128-chunk 2h+u
            x8_s = [single([P, 2 * T], F8, f"x8s{h}") for h in range(2)]
            w8_s = [single([P, 2 * WPK], F8, f"w8s{h}") for h in range(2)]
            bp_s = single([P, BPK], F32, "bps")
            # packed layout: bq[0:4] | bk[4:8] | bv[8:520] | mask[520:648]
            qt8_s = [single([P, 2 * T], F8, f"qt8s{h}") for h in range(2)]
            kt8_s = [single([P, 2 * T], F8, f"kt8s{h}") for h in range(2)]
            v16_s = [single([P, VDIM], F16, f"v16s{t}") for t in range(NTC)]
            v8_s = [single([P, 2 * VDIM], F8, f"v8s{m}") for m in range(NTC // 2)]
            # P~ fp8 pair tiles: pair m holds strips jc=2m,2m+1; valid
            # i >= a0 = 512*(m//2); plane length Lm = T - a0
            pt8_s = []
            for m in range(NTC // 2):
                Lm = T - 512 * (m // 2)
                pt8_s.append(single([P, 2 * Lm], F8, f"pt8s{m}"))
            zr_s = single([P, NTC], F32, "zrs")
            expshift_s = single([P, 1], F32, "expshift")
            nc.vector.memset(expshift_s, EXP_SHIFT)

            # ---- input DMAs: triggers cost ~610ns each and serialize per
            # engine queue, so spread them (Scalar: weights, Tensor: x8,
            # GpSimd: x copy-through) to start all transfers by ~8us ----
            nc.scalar.dma_start(out=bp_s, in_=bp_d[:, :])
            for h in range(2):
                for u in range(2):
                    cc = 2 * h + u
                    nc.scalar.dma_start(
                        out=w8_s[h][:, _ts(u, WPK)], in_=w8_d[_ts(cc, P), :]
                    )
            for h in range(2):
                for u in range(2):
                    cc = 2 * h + u
                    nc.sync.dma_start(
                        out=x8_s[h][:, _ts(u, T)], in_=x8_d[_ts(cc, P), :]
                    )
            # x copy-through rows 0..511 (DRAM->DRAM), overlaps everything
            for c in range(4):
                nc.gpsimd.dma_start(out=out_d[_ts(c, P), :], in_=x32_d[_ts(c, P), :])

            # ---- Phase QKV: Q, K projections (fp8 DoubleRow) ----
            qkv_ps_cm = tc.tile_pool(name="qkv_ps", bufs=8, space="PSUM")
            qkv_ps = qkv_ps_cm.__enter__()
            for wofs, bofs, dst in ((0, 0, qt8_s), (KDIM, 4, kt8_s)):
                for g in range(2):
                    for kk in range(4):
                        pss = [
                            qkv_ps.tile([P, 512], F32, tag="qkvps", name="ps_qk")
                            for _ in range(4)
                        ]
                        for h in range(2):
                            lhs3 = re2(w8_s[h])[:, :, wofs + kk * P : wofs + (kk + 1) * P]
                            for gi in range(4):
                                ib = 4 * g + gi
                                nc.tensor.matmul(
                                    pss[gi],
                                    lhsT=lhs3,
                                    rhs=re2(x8_s[h])[:, :, _ts(ib, 512)],
                                    start=(h == 0),
                                    stop=(h == 1),
                                    perf_mode=DR,
                                )
                        for gi in range(4):
                            ib = 4 * g + gi
                            dst_ap = dst[kk // 2][
                                :, (kk % 2) * T + ib * 512 : (kk % 2) * T + ib * 512 + 512
                            ]
                            # evictions pace QKV: split across DVE/ScalarE
                            if gi % 2 == 0:
                                nc.vector.tensor_scalar(
                                    dst_ap,
                                    pss[gi],
                                    1.0 / C4,
                                    bp_s[:, bofs + kk : bofs + kk + 1],
                                    op0=mybir.AluOpType.mult,
                                    op1=mybir.AluOpType.add,
                                )
                            else:
                                nc.scalar.activation(
                                    dst_ap,
                                    pss[gi],
                                    mybir.ActivationFunctionType.Identity,
                                    bias=bp_s[:, bofs + kk : bofs + kk + 1],
                                    scale=1.0 / C4,
                                )
            qkv_ps_cm.__exit__(None, None, None)

            # ---- Phase 1 + 2 interleaved ----
            # s_ps: [128,1024] f32 tiles (2 PSUM banks), bufs=2 -> 4 banks
            # act_ps: 4 x [128,512] (1 bank each) -> 4 banks
            s_ps = ctx.enter_context(tc.tile_pool(name="s_ps", bufs=2, space="PSUM"))
            act_ps = ctx.enter_context(
                tc.tile_pool(name="act_ps", bufs=1, space="PSUM")
            )
            zp_pool = ctx.enter_context(tc.tile_pool(name="zp", bufs=4))
            vt_pool = ctx.enter_context(tc.tile_pool(name="vt", bufs=4))
            ob_pool = ctx.enter_context(tc.tile_pool(name="ob", bufs=4))

            # work queue of deferred act-block ops (closures), pumped
            # between strip chunks so TensorE never idles on the exp chain
            pending = []

            def pump(n):
                for _ in range(min(n, len(pending))):
                    pending.pop(0)()

            def emit_v_tile(t):
                # V projection tile t; borrows an s_ps slot (uses half)
                ps = s_ps.tile([P, 1024], F32, tag="sps", name="ps_v")
                for h in range(2):
                    nc.tensor.matmul(
                        ps[:, 0:512],
                        lhsT=re2(x8_s[h])[:, :, _ts(t, P)],
                        rhs=re2(w8_s[h])[:, :, 2 * KDIM : 3 * KDIM],
                        start=(h == 0),
                        stop=(h == 1),
                        perf_mode=DR,
                    )
                nc.vector.tensor_add(v16_s[t], ps[:, 0:512], bp_s[:, 8 : 8 + VDIM])

            def enqueue_act_block(ib):
                nm = 2 * (ib + 1)  # pairs m contributing to block ib
                pss = [
                    act_ps.tile([P, 512], F32, tag=f"aps{v}", name=f"aps{v}")
                    for v in range(4)
                ]

                def mk_mm(m, vc):
                    def go():
                        off = 512 * ib - 512 * (m // 2)
                        nc.tensor.matmul(
                            pss[vc],
                            lhsT=re2(v8_s[m])[:, :, _ts(vc, P)],
                            rhs=re2(pt8_s[m])[:, :, off : off + 512],
                            start=(m == 0),
                            stop=(m == nm - 1),
                            perf_mode=DR,
                        )

                    return go

                def mk_ev(vc):
                    def go():
                        ob = ob_pool.tile([P, 512], F32, tag="ob", name="ob")
                        nc.vector.tensor_copy(ob, pss[vc])
                        nc.sync.dma_start(
                            out=out_d[C + vc * P : C + (vc + 1) * P, _ts(ib, 512)],
                            in_=ob,
                        )

                    return go

                for m in range(nm):
                    for vc in range(4):
                        pending.append(mk_mm(m, vc))
                for vc in range(4):
                    pending.append(mk_ev(vc))

            for t in range(4):
                emit_v_tile(t)

            for jc in range(NTC):
                i0 = P * jc
                a0 = 512 * (jc // 4)
                m, u = jc // 2, jc % 2
                Lm = T - a0
                if jc + 4 < NTC:
                    emit_v_tile(jc + 4)
                r = jc % 4
                if r > 0:
                    # zero the never-written corner [a0, i0)
                    nc.vector.memset(pt8_s[m][:, u * Lm : u * Lm + P * r], 0.0)
                starts = [i0] + list(range(a0 + 1024, T, 1024))
                nch = len(starts)
                zp = zp_pool.tile([P, 4], F32, tag="zp", name="zp")
                for ci, a in enumerate(starts):
                    b = min(a0 + 1024 * (ci + 1), T)
                    w = b - a
                    ps = s_ps.tile([P, 1024], F32, tag="sps", name="ps_s")
                    segs = [(0, min(w, 512))]
                    if w > 512:
                        segs.append((512, w))
                    for h in range(2):
                        lhs3 = re2(kt8_s[h])[:, :, i0 : i0 + P]
                        for s0, s1 in segs:
                            nc.tensor.matmul(
                                ps[:, s0:s1],
                                lhsT=lhs3,
                                rhs=re2(qt8_s[h])[:, :, a + s0 : a + s1],
                                start=(h == 0),
                                stop=(h == 1),
                                perf_mode=DR,
                            )
                    if ci == 0:
                        nc.vector.tensor_add(
                            ps[:, 0:P], ps[:, 0:P], bp_s[:, 8 + VDIM : 8 + VDIM + P]
                        )
                    nc.scalar.activation(
                        pt8_s[m][:, u * Lm + (a - a0) : u * Lm + (b - a0)],
                        ps[:, 0:w],
                        mybir.ActivationFunctionType.Exp,
                        bias=expshift_s[:, 0:1],
                        scale=1.0,
                        accum_out=zp[:, ci : ci + 1],
                    )
                    pump(5)
                z = zp_pool.tile([P, 1], F32, tag="zf", name="z")
                nc.vector.reduce_sum(z, zp[:, 0:nch], axis=mybir.AxisListType.X)
                nc.vector.reciprocal(zr_s[:, jc : jc + 1], z)
                # fold 1/Z_j into V rows; clip +-240 (fp8e4 overflows to
                # inf/NaN, no saturation) then cast fp8 into pair plane
                # on GpSimd: keeps DVE free so the next strip's mask-add
                # doesn't queue behind this chain
                vt = vt_pool.tile([P, VDIM], F16, tag="vt", name="vt")
                nc.gpsimd.tensor_scalar(
                    vt,
                    v16_s[jc],
                    zr_s[:, jc : jc + 1],
                    FP8MAX,
                    op0=mybir.AluOpType.mult,
                    op1=mybir.AluOpType.min,
                )
                nc.gpsimd.tensor_scalar_max(v8_s[m][:, _ts(u, VDIM)], vt, -FP8MAX)
                if jc % 4 == 3:
                    enqueue_act_block(jc // 4)

            while pending:
                pump(len(pending))

    nc.compile()
    return nc


def _host_inputs(x, Wq, bq, Wk, bk, Wv, bv):
    import ml_dtypes

    def f8(a):
        return np.clip(a, -FP8MAX, FP8MAX).astype(ml_dtypes.float8_e4m3)

    w8 = f8(np.concatenate([Wq.T, Wk.T, Wv.T], axis=1))  # [C, 1536]
    r = np.arange(P)
    mask = np.where(r[None, :] >= r[:, None], 0.0, MASK_NEG).astype(np.float32)
    bp = np.concatenate(
        [
            (bq / C4).reshape(4, P).T,
            (bk / C4).reshape(4, P).T,
            np.tile(bv.astype(np.float32), (P, 1)),
            mask,
        ],
        axis=1,
    ).astype(np.float32)
    bp = np.ascontiguousarray(bp)
    in_maps = []
    for b in range(x.shape[0]):
        xb = np.ascontiguousarray(x[b]).astype(np.float32)
        in_maps.append({"x8": f8(xb), "x32": xb, "w8": w8, "bp": bp})
    return in_maps


def kernel(x, Wq, bq, Wk, bk, Wv, bv, _trace=False, _tmpdir=None):
    import time as _time

    x = np.asarray(x, dtype=np.float32)
    if "nc" not in _CACHE:
        t0 = _time.time()
        _CACHE["nc"] = build_nc()
        print(f"[kernel] build_nc done in {_time.time() - t0:.1f}s", flush=True)
    nc = _CACHE["nc"]
    in_maps = _host_inputs(
        x,
        np.asarray(Wq, np.float32),
        np.asarray(bq, np.float32),
        np.asarray(Wk, np.float32),
        np.asarray(bk, np.float32),
        np.asarray(Wv, np.float32),
        np.asarray(bv, np.float32),
    )
    t0 = _time.time()
    res = run_bass_kernel_spmd(
        nc, in_maps, core_ids=list(range(8)), trace=_trace, tmpdir=_tmpdir
    )
    print(f"[kernel] run done in {_time.time() - t0:.1f}s", flush=True)
    _CACHE["last_result"] = res
    out = np.stack([r["out"] for r in res.results]).astype(np.float32)
    return out


# revision 19
# speedup vs baseline: 2.6898x; 2.6898x over previous
"""Trainium2 Bass kernel for nn_AttentionBlock (sparse_attention).

Reference computation per batch b (channels-first x[b]: [C=512, T=4096]):
    xt = x[b].T                                  # [T, C]
    q = xt @ Wq.T + bq ; k = xt @ Wk.T + bk      # [T, 512]
    v = xt @ Wv.T + bv                           # [T, 512]
    S = q @ k.T / sqrt(512), causal (j <= i)     # [T, T]
    P = softmax(S, axis=QUERY i)  (per-column normalization)
    act = P @ v                                  # [T, 512]
    out[b] = concat(x[b], act.T, axis=0)         # [1024, T]

Sharding: pure data-parallel over batch B=8 across the 8 NeuronCores
(one batch per core, no collectives).

v3 per-core algorithm — all matmuls fp8e4 DoubleRow (f32 PSUM):
  1. QKV projections from x8 (host-cast fp8).  Contraction over C=512
     as 2 DoubleRow pairs.  Q^T,K^T stored fp8 paired over head-dim
     chunks; V rows fp16.  Q/K eviction on DVE: (psum/c4) + b/c4 ->
     fp8 (c4 = 512**0.25 splits the score scale between q and k).
     Weights+biases+mask ride in 2 packed DRAM params (DMA triggers
     cost ~650ns each on the Sync engine; fewer, bigger transfers).
  2. Score strips ST[j,i] = K^T.T @ Q^T (j-chunk of 128 keys, i from
     the diagonal to T in 1024-col PSUM chunks; 512-col DoubleRow MMs,
     h-outer so LDWEIGHTS amortizes 2 MMs).  Column softmax over i:
     additive causal mask on the diagonal block, exp(s-4) on ScalarE
     with accum_out producing Z_j partials (1024-col chunks halve the
     per-instruction ACTIVATE/READ_ACCUMULATOR overhead).  P~ stays in
     SBUF as fp8 pair tiles (triangle = 72KB/partition).
  3. V rows scaled by 1/Z_j on DVE, clipped to +-240 (fp8e4 overflows
     to inf/NaN, no saturation), cast fp8 into paired v8 tiles.
  4. act^T[v,i] = sum_j V'[j,v] * P~[j,i]: PSUM-accumulated DoubleRow
     matmuls from SBUF, interleaved between score-strip chunks so
     TensorE never stalls on the ScalarE exp chain.  DVE eviction.
  5. out rows 0..511 are a DRAM->DRAM copy of x[b] overlapping the
     whole kernel.
"""

import math

import numpy as np

import concourse.bass as bass
import concourse.mybir as mybir
from concourse import bacc, tile
from concourse.bass_utils import run_bass_kernel_spmd

P = 128
C = 512
T = 4096
KDIM = 512
VDIM = 512
NTC = T // P      # 32 time chunks of 128
NIB = T // 512    # 8 i-blocks of 512
F16 = mybir.dt.float16
F32 = mybir.dt.float32
F8 = mybir.dt.float8e4
EXP_SHIFT = -4.0  # constant logit shift: softmax-invariant, keeps exp in range
MASK_NEG = -10000.0
C4 = float(C) ** 0.25
FP8MAX = 240.0
DR = mybir.MatmulPerfMode.DoubleRow
WPK = 3 * KDIM    # packed weight row: wq | wk | wv
BPK = 4 + 4 + VDIM + P  # packed per-partition consts: bq | bk | bv | mask

_CACHE = {}


def _ts(i, size):
    return slice(i * size, (i + 1) * size)


def build_nc():
    nc = bacc.Bacc(
        "TRN2",
        target_bir_lowering=False,
        debug=False,
        num_devices=8,
    )

    x8_d = nc.declare_dram_parameter("x8", [C, T], F8, isOutput=False)
    x32_d = nc.declare_dram_parameter("x32", [C, T], F32, isOutput=False)
    w8_d = nc.declare_dram_parameter("w8", [C, WPK], F8, isOutput=False)
    bp_d = nc.declare_dram_parameter("bp", [P, BPK], F32, isOutput=False)
    out_d = nc.declare_dram_parameter("out", [C + VDIM, T], F32, isOutput=True)

    def re2(ap):
        return ap.rearrange("p (u n) -> p u n", u=2)

    with tile.TileContext(nc) as tc:
        from contextlib import ExitStack

        with ExitStack() as ctx:
            singles = ctx.enter_context(tc.tile_pool(name="singles", bufs=1))

            def single(shape, dtype, tag):
                return singles.tile(shape, dtype, name=tag, tag=tag)

            # paired fp8 layouts: plane u of tile h holds 128-chunk 2h+u
            x8_s = [single([P, 2 * T], F8, f"x8s{h}") for h in range(2)]
            w8_s = [single([P, 2 * WPK], F8, f"w8s{h}") for h in range(2)]
            bp_s = single([P, BPK], F32, "bps")
            # packed layout: bq[0:4] | bk[4:8] | bv[8:520] | mask[520:648]
            qt8_s = [single([P, 2 * T], F8, f"qt8s{h}") for h in range(2)]
            kt8_s = [single([P, 2 * T], F8, f"kt8s{h}") for h in range(2)]
            v16_s = [single([P, VDIM], F16, f"v16s{t}") for t in range(NTC)]
            v8_s = [single([P, 2 * VDIM], F8, f"v8s{m}") for m in range(NTC // 2)]
            # P~ fp8 pair tiles: pair m holds strips jc=2m,2m+1; valid
            # i >= a0 = 512*(m//2); plane length Lm = T - a0
            pt8_s = []
            for m in range(NTC // 2):
                Lm = T - 512 * (m // 2)
                pt8_s.append(single([P, 2 * Lm], F8, f"pt8s{m}"))
            zr_s = single([P, NTC], F32, "zrs")
            expshift_s = single([P, 1], F32, "expshift")
            nc.vector.memset(expshift_s, EXP_SHIFT)

            # ---- input DMAs: triggers cost ~610ns each and serialize per
            # engine queue, so spread them (Scalar: weights, Tensor: x8,
            # GpSimd: x copy-through) to start all transfers by ~8us ----
            nc.scalar.dma_start(out=bp_s, in_=bp_d[:, :])
            for h in range(2):
                for u in range(2):
                    cc = 2 * h + u
                    nc.scalar.dma_start(
                        out=w8_s[h][:, _ts(u, WPK)], in_=w8_d[_ts(cc, P), :]
                    )
            for h in range(2):
                for u in range(2):
                    cc = 2 * h + u
                    nc.sync.dma_start(
                        out=x8_s[h][:, _ts(u, T)], in_=x8_d[_ts(cc, P), :]
                    )
            # x copy-through rows 0..511 (DRAM->DRAM), overlaps everything
            for c in range(4):
                nc.gpsimd.dma_start(out=out_d[_ts(c, P), :], in_=x32_d[_ts(c, P), :])

            # ---- Phase QKV: Q, K projections (fp8 DoubleRow) ----
            qkv_ps_cm = tc.tile_pool(name="qkv_ps", bufs=8, space="PSUM")
            qkv_ps = qkv_ps_cm.__enter__()
            for wofs, bofs, dst in ((0, 0, qt8_s), (KDIM, 4, kt8_s)):
                for g in range(2):
                    for kk in range(4):
                        pss = [
                            qkv_ps.tile([P, 512], F32, tag="qkvps", name="ps_qk")
                            for _ in range(4)
                        ]
                        for h in range(2):
                            lhs3 = re2(w8_s[h])[:, :, wofs + kk * P : wofs + (kk + 1) * P]
                            for gi in range(4):
                                ib = 4 * g + gi
                                nc.tensor.matmul(
                                    pss[gi],
                                    lhsT=lhs3,
                                    rhs=re2(x8_s[h])[:, :, _ts(ib, 512)],
                                    start=(h == 0),
                                    stop=(h == 1),
                                    perf_mode=DR,
                                )
                        for gi in range(4):
                            ib = 4 * g + gi
                            dst_ap = dst[kk // 2][
                                :, (kk % 2) * T + ib * 512 : (kk % 2) * T + ib * 512 + 512
                            ]
                            # evictions pace QKV: split across DVE/ScalarE
                            if gi % 2 == 0:
                                nc.vector.tensor_scalar(
                                    dst_ap,
                                    pss[gi],
                                    1.0 / C4,
                                    bp_s[:, bofs + kk : bofs + kk + 1],
                                    op0=mybir.AluOpType.mult,
                                    op1=mybir.AluOpType.add,
                                )
                            else:
                                nc.scalar.activation(
                                    dst_ap,
                                    pss[gi],
                                    mybir.ActivationFunctionType.Identity,
                                    bias=bp_s[:, bofs + kk : bofs + kk + 1],
                                    scale=1.0 / C4,
                                )
            qkv_ps_cm.__exit__(None, None, None)

            # ---- Phase 1 + 2 interleaved ----
            # s_ps: [128,1024] f32 tiles (2 PSUM banks), bufs=2 -> 4 banks
            # act_ps: 4 x [128,512] (1 bank each) -> 4 banks
            s_ps = ctx.enter_context(tc.tile_pool(name="s_ps", bufs=2, space="PSUM"))
            act_ps = ctx.enter_context(
                tc.tile_pool(name="act_ps", bufs=1, space="PSUM")
            )
            zp_pool = ctx.enter_context(tc.tile_pool(name="zp", bufs=4))
            vt_pool = ctx.enter_context(tc.tile_pool(name="vt", bufs=4))
            ob_pool = ctx.enter_context(tc.tile_pool(name="ob", bufs=4))

            # work queue of deferred act-block ops (closures), pumped
            # between strip chunks so TensorE never idles on the exp chain
            pending = []

            def pump(n):
                for _ in range(min(n, len(pending))):
                    pending.pop(0)()

            def emit_v_tile(t):
                # V projection tile t; borrows an s_ps slot (uses half)
                ps = s_ps.tile([P, 1024], F32, tag="sps", name="ps_v")
                for h in range(2):
                    nc.tensor.matmul(
                        ps[:, 0:512],
                        lhsT=re2(x8_s[h])[:, :, _ts(t, P)],
                        rhs=re2(w8_s[h])[:, :, 2 * KDIM : 3 * KDIM],
                        start=(h == 0),
                        stop=(h == 1),
                        perf_mode=DR,
                    )
                nc.vector.tensor_add(v16_s[t], ps[:, 0:512], bp_s[:, 8 : 8 + VDIM])

            def enqueue_act_block(ib):
                nm = 2 * (ib + 1)  # pairs m contributing to block ib
                pss = [
                    act_ps.tile([P, 512], F32, tag=f"aps{v}", name=f"aps{v}")
                    for v in range(4)
                ]

                def mk_mm(m, vc):
                    def go():
                        off = 512 * ib - 512 * (m // 2)
                        nc.tensor.matmul(
                            pss[vc],
                            lhsT=re2(v8_s[m])[:, :, _ts(vc, P)],
                            rhs=re2(pt8_s[m])[:, :, off : off + 512],
                            start=(m == 0),
                            stop=(m == nm - 1),
                            perf_mode=DR,
                        )

                    return go

                def mk_ev(vc):
                    def go():
                        ob = ob_pool.tile([P, 512], F32, tag="ob", name="ob")
                        nc.vector.tensor_copy(ob, pss[vc])
                        nc.sync.dma_start(
                            out=out_d[C + vc * P : C + (vc + 1) * P, _ts(ib, 512)],
                            in_=ob,
                        )

                    return go

                for m in range(nm):
                    for vc in range(4):
                        pending.append(mk_mm(m, vc))
                for vc in range(4):
                    pending.append(mk_ev(vc))

            for t in range(4):
                emit_v_tile(t)

            for jc in range(NTC):
                i0 = P * jc
                a0 = 512 * (jc // 4)
                m, u = jc // 2, jc % 2
                Lm = T - a0
                if jc + 4 < NTC:
                    emit_v_tile(jc + 4)
                r = jc % 4
                if r > 0:
                    # zero the never-written corner [a0, i0)
                    nc.vector.memset(pt8_s[m][:, u * Lm : u * Lm + P * r], 0.0)
                starts = [i0] + list(range(a0 + 1024, T, 1024))
                nch = len(starts)
                zp = zp_pool.tile([P, 4], F32, tag="zp", name="zp")
                for ci, a in enumerate(starts):
                    b = min(a0 + 1024 * (ci + 1), T)
                    w = b - a
                    ps = s_ps.tile([P, 1024], F32, tag="sps", name="ps_s")
                    segs = [(0, min(w, 512))]
                    if w > 512:
                        segs.append((512, w))
                    for h in range(2):
                        lhs3 = re2(kt8_s[h])[:, :, i0 : i0 + P]
                        for s0, s1 in segs:
                            nc.tensor.matmul(
                                ps[:, s0:s1],
                                lhsT=lhs3,
                                rhs=re2(qt8_s[h])[:, :, a + s0 : a + s1],
                                start=(h == 0),
                                stop=(h == 1),
                                perf_mode=DR,
                            )
                    if ci == 0:
                        nc.vector.tensor_add(
                            ps[:, 0:P], ps[:, 0:P], bp_s[:, 8 + VDIM : 8 + VDIM + P]
                        )
                    nc.scalar.activation(
                        pt8_s[m][:, u * Lm + (a - a0) : u * Lm + (b - a0)],
                        ps[:, 0:w],
                        mybir.ActivationFunctionType.Exp,
                        bias=expshift_s[:, 0:1],
                        scale=1.0,
                        accum_out=zp[:, ci : ci + 1],
                    )
                    pump(5)
                z = zp_pool.tile([P, 1], F32, tag="zf", name="z")
                nc.vector.reduce_sum(z, zp[:, 0:nch], axis=mybir.AxisListType.X)
                nc.vector.reciprocal(zr_s[:, jc : jc + 1], z)
                # fold 1/Z_j into V rows; clip +-240 (fp8e4 overflows to
                # inf/NaN, no saturation) then cast fp8 into pair plane
                vt = vt_pool.tile([P, VDIM], F16, tag="vt", name="vt")
                nc.vector.tensor_scalar(
                    vt,
                    v16_s[jc],
                    zr_s[:, jc : jc + 1],
                    FP8MAX,
                    op0=mybir.AluOpType.mult,
                    op1=mybir.AluOpType.min,
                )
                nc.vector.tensor_scalar_max(v8_s[m][:, _ts(u, VDIM)], vt, -FP8MAX)
                if jc % 4 == 3:
                    enqueue_act_block(jc // 4)

            while pending:
                pump(len(pending))

    nc.compile()
    return nc


def _host_inputs(x, Wq, bq, Wk, bk, Wv, bv):
    import ml_dtypes

    def f8(a):
        return np.clip(a, -FP8MAX, FP8MAX).astype(ml_dtypes.float8_e4m3)

    w8 = f8(np.concatenate([Wq.T, Wk.T, Wv.T], axis=1))  # [C, 1536]
    r = np.arange(P)
    mask = np.where(r[None, :] >= r[:, None], 0.0, MASK_NEG).astype(np.float32)
    bp = np.concatenate(
        [
            (bq / C4).reshape(4, P).T,
            (bk / C4).reshape(4, P).T,
            np.tile(bv.astype(np.float32), (P, 1)),
            mask,
        ],
        axis=1,
    ).astype(np.float32)
    bp = np.ascontiguousarray(bp)
    in_maps = []
    for b in range(x.shape[0]):
        xb = np.ascontiguousarray(x[b]).astype(np.float32)
        in_maps.append({"x8": f8(xb), "x32": xb, "w8": w8, "bp": bp})
    return in_maps


def kernel(x, Wq, bq, Wk, bk, Wv, bv, _trace=False, _tmpdir=None):
    import time as _time

    x = np.asarray(x, dtype=np.float32)
    if "nc" not in _CACHE:
        t0 = _time.time()
        _CACHE["nc"] = build_nc()
        print(f"[kernel] build_nc done in {_time.time() - t0:.1f}s", flush=True)
    nc = _CACHE["nc"]
    in_maps = _host_inputs(
        x,
        np.asarray(Wq, np.float32),
        np.asarray(bq, np.float32),
        np.asarray(Wk, np.float32),
        np.asarray(bk, np.float32),
        np.asarray(Wv, np.float32),
        np.asarray(bv, np.float32),
    )
    t0 = _time.time()
    res = run_bass_kernel_spmd(
        nc, in_maps, core_ids=list(range(8)), trace=_trace, tmpdir=_tmpdir
    )
    print(f"[kernel] run done in {_time.time() - t0:.1f}s", flush=True)
    _CACHE["last_result"] = res
    out = np.stack([r["out"] for r in res.results]).astype(np.float32)
    return out


# revision 25
# speedup vs baseline: 2.9881x; 1.1109x over previous
"""Trainium2 Bass kernel for nn_AttentionBlock (sparse_attention).

Reference computation per batch b (channels-first x[b]: [C=512, T=4096]):
    xt = x[b].T                                  # [T, C]
    q = xt @ Wq.T + bq ; k = xt @ Wk.T + bk      # [T, 512]
    v = xt @ Wv.T + bv                           # [T, 512]
    S = q @ k.T / sqrt(512), causal (j <= i)     # [T, T]
    P = softmax(S, axis=QUERY i)  (per-column normalization)
    act = P @ v                                  # [T, 512]
    out[b] = concat(x[b], act.T, axis=0)         # [1024, T]

Sharding: pure data-parallel over batch B=8 across the 8 NeuronCores
(one batch per core, no collectives).

v3 per-core algorithm — all matmuls fp8e4 DoubleRow (f32 PSUM):
  1. QKV projections from x8 (host-cast fp8).  Contraction over C=512
     as 2 DoubleRow pairs.  Q^T,K^T stored fp8 paired over head-dim
     chunks; V rows fp16.  Q/K eviction on DVE: (psum/c4) + b/c4 ->
     fp8 (c4 = 512**0.25 splits the score scale between q and k).
     Weights+biases+mask ride in 2 packed DRAM params (DMA triggers
     cost ~650ns each on the Sync engine; fewer, bigger transfers).
  2. Score strips ST[j,i] = K^T.T @ Q^T (j-chunk of 128 keys, i from
     the diagonal to T in 1024-col PSUM chunks; 512-col DoubleRow MMs,
     h-outer so LDWEIGHTS amortizes 2 MMs).  Column softmax over i:
     additive causal mask on the diagonal block, exp(s-4) on ScalarE
     with accum_out producing Z_j partials (1024-col chunks halve the
     per-instruction ACTIVATE/READ_ACCUMULATOR overhead).  P~ stays in
     SBUF as fp8 pair tiles (triangle = 72KB/partition).
  3. V rows scaled by 1/Z_j on DVE, clipped to +-240 (fp8e4 overflows
     to inf/NaN, no saturation), cast fp8 into paired v8 tiles.
  4. act^T[v,i] = sum_j V'[j,v] * P~[j,i]: PSUM-accumulated DoubleRow
     matmuls from SBUF, interleaved between score-strip chunks so
     TensorE never stalls on the ScalarE exp chain.  DVE eviction.
  5. out rows 0..511 are a DRAM->DRAM copy of x[b] overlapping the
     whole kernel.
"""

import math

import numpy as np

import concourse.bass as bass
import concourse.mybir as mybir
from concourse import bacc, tile
from concourse.bass_utils import run_bass_kernel_spmd

P = 128
C = 512
T = 4096
KDIM = 512
VDIM = 512
NTC = T // P      # 32 time chunks of 128
NIB = T // 512    # 8 i-blocks of 512
F16 = mybir.dt.float16
F32 = mybir.dt.float32
F8 = mybir.dt.float8e4
EXP_SHIFT = -4.0  # constant logit shift: softmax-invariant, keeps exp in range
MASK_NEG = -10000.0
C4 = float(C) ** 0.25
FP8MAX = 240.0
DR = mybir.MatmulPerfMode.DoubleRow
WPK = 3 * KDIM    # packed weight row: wq | wk | wv
BPK = 4 + 4 + VDIM + P  # packed per-partition consts: bq | bk | bv | mask

_CACHE = {}


def _ts(i, size):
    return slice(i * size, (i + 1) * size)


def build_nc():
    nc = bacc.Bacc(
        "TRN2",
        target_bir_lowering=False,
        debug=False,
        num_devices=8,
    )

    x8_d = nc.declare_dram_parameter("x8", [C, T], F8, isOutput=False)
    x32_d = nc.declare_dram_parameter("x32", [C, T], F32, isOutput=False)
    w8_d = nc.declare_dram_parameter("w8", [C, WPK], F8, isOutput=False)
    bp_d = nc.declare_dram_parameter("bp", [P, BPK], F32, isOutput=False)
    out_d = nc.declare_dram_parameter("out", [C + VDIM, T], F32, isOutput=True)

    def re2(ap):
        return ap.rearrange("p (u n) -> p u n", u=2)

    with tile.TileContext(nc) as tc:
        from contextlib import ExitStack

        with ExitStack() as ctx:
            singles = ctx.enter_context(tc.tile_pool(name="singles", bufs=1))

            def single(shape, dtype, tag):
                return singles.tile(shape, dtype, name=tag, tag=tag)

            # paired fp8 layouts: plane u of tile h holds 128-chunk 2h+u
            x8_s = [single([P, 2 * T], F8, f"x8s{h}") for h in range(2)]
            w8_s = [single([P, 2 * WPK], F8, f"w8s{h}") for h in range(2)]
            bp_s = single([P, BPK], F32, "bps")
            # packed layout: bq[0:4] | bk[4:8] | bv[8:520] | mask[520:648]
            qt8_s = [single([P, 2 * T], F8, f"qt8s{h}") for h in range(2)]
            kt8_s = [single([P, 2 * T], F8, f"kt8s{h}") for h in range(2)]
            v16_s = [single([P, VDIM], F16, f"v16s{t}") for t in range(NTC)]
            v8_s = [single([P, 2 * VDIM], F8, f"v8s{m}") for m in range(NTC // 2)]
            # P~ fp8 pair tiles: pair m holds strips jc=2m,2m+1; valid
            # i >= a0 = 512*(m//2); plane length Lm = T - a0
            pt8_s = []
            for m in range(NTC // 2):
                Lm = T - 512 * (m // 2)
                pt8_s.append(single([P, 2 * Lm], F8, f"pt8s{m}"))
            zr_s = single([P, NTC], F32, "zrs")
            expshift_s = single([P, 1], F32, "expshift")
            nc.vector.memset(expshift_s, EXP_SHIFT)

            # ---- input DMAs: triggers cost ~610ns each and serialize on
            # the Sync queue, so batch via 3D APs (both u-planes per DMA)
            # and put compute-critical transfers first ----
            for h in range(2):
                nc.sync.dma_start(
                    out=re2(w8_s[h]),
                    in_=w8_d[2 * h * P : (2 * h + 2) * P, :].rearrange(
                        "(u p) n -> p u n", u=2
                    ),
                )
            # x8 in two column pieces so the first QKV group unblocks early
            for piece in range(2):
                for h in range(2):
                    nc.sync.dma_start(
                        out=re2(x8_s[h])[:, :, _ts(piece, 2048)],
                        in_=x8_d[
                            2 * h * P : (2 * h + 2) * P, _ts(piece, 2048)
                        ].rearrange("(u p) n -> p u n", u=2),
                    )
                if piece == 0:
                    nc.sync.dma_start(out=bp_s, in_=bp_d[:, :])
            # x copy-through rows 0..511 (DRAM->DRAM), overlaps everything
            nc.sync.dma_start(out=out_d[0:C, :], in_=x32_d[:, :])

            # ---- Phase QKV: Q, K projections (fp8 DoubleRow) ----
            qkv_ps_cm = tc.tile_pool(name="qkv_ps", bufs=8, space="PSUM")
            qkv_ps = qkv_ps_cm.__enter__()
            for wofs, bofs, dst in ((0, 0, qt8_s), (KDIM, 4, kt8_s)):
                for g in range(2):
                    for kk in range(4):
                        pss = [
                            qkv_ps.tile([P, 512], F32, tag="qkvps", name="ps_qk")
                            for _ in range(4)
                        ]
                        for h in range(2):
                            lhs3 = re2(w8_s[h])[:, :, wofs + kk * P : wofs + (kk + 1) * P]
                            for gi in range(4):
                                ib = 4 * g + gi
                                nc.tensor.matmul(
                                    pss[gi],
                                    lhsT=lhs3,
                                    rhs=re2(x8_s[h])[:, :, _ts(ib, 512)],
                                    start=(h == 0),
                                    stop=(h == 1),
                                    perf_mode=DR,
                                )
                        for gi in range(4):
                            ib = 4 * g + gi
                            dst_ap = dst[kk // 2][
                                :, (kk % 2) * T + ib * 512 : (kk % 2) * T + ib * 512 + 512
                            ]
                            # evictions pace QKV: split across DVE/ScalarE
                            if gi % 2 == 0:
                                nc.vector.tensor_scalar(
                                    dst_ap,
                                    pss[gi],
                                    1.0 / C4,
                                    bp_s[:, bofs + kk : bofs + kk + 1],
                                    op0=mybir.AluOpType.mult,
                                    op1=mybir.AluOpType.add,
                                )
                            else:
                                nc.scalar.activation(
                                    dst_ap,
                                    pss[gi],
                                    mybir.ActivationFunctionType.Identity,
                                    bias=bp_s[:, bofs + kk : bofs + kk + 1],
                                    scale=1.0 / C4,
                                )
            qkv_ps_cm.__exit__(None, None, None)

            # ---- Phase 1 + 2 interleaved ----
            # s_ps: [128,1024] f32 tiles (2 PSUM banks), bufs=2 -> 4 banks
            # act_ps: 4 x [128,512] (1 bank each) -> 4 banks
            s_ps = ctx.enter_context(tc.tile_pool(name="s_ps", bufs=2, space="PSUM"))
            act_ps = ctx.enter_context(
                tc.tile_pool(name="act_ps", bufs=1, space="PSUM")
            )
            zp_pool = ctx.enter_context(tc.tile_pool(name="zp", bufs=4))
            vt_pool = ctx.enter_context(tc.tile_pool(name="vt", bufs=4))
            ob_pool = ctx.enter_context(tc.tile_pool(name="ob", bufs=2))

            # work queue of deferred act-block ops (closures), pumped
            # between strip chunks so TensorE never idles on the exp chain
            pending = []

            def pump(n):
                for _ in range(min(n, len(pending))):
                    pending.pop(0)()

            def emit_v_tile(t):
                # V projection tile t; borrows an s_ps slot (uses half)
                ps = s_ps.tile([P, 1024], F32, tag="sps", name="ps_v")
                for h in range(2):
                    nc.tensor.matmul(
                        ps[:, 0:512],
                        lhsT=re2(x8_s[h])[:, :, _ts(t, P)],
                        rhs=re2(w8_s[h])[:, :, 2 * KDIM : 3 * KDIM],
                        start=(h == 0),
                        stop=(h == 1),
                        perf_mode=DR,
                    )
                nc.vector.tensor_add(v16_s[t], ps[:, 0:512], bp_s[:, 8 : 8 + VDIM])

            def enqueue_act_block(ib):
                nm = 2 * (ib + 1)  # pairs m contributing to block ib
                pss = [
                    act_ps.tile([P, 512], F32, tag=f"aps{v}", name=f"aps{v}")
                    for v in range(4)
                ]

                def mk_mm(m, vc):
                    def go():
                        off = 512 * ib - 512 * (m // 2)
                        nc.tensor.matmul(
                            pss[vc],
                            lhsT=re2(v8_s[m])[:, :, _ts(vc, P)],
                            rhs=re2(pt8_s[m])[:, :, off : off + 512],
                            start=(m == 0),
                            stop=(m == nm - 1),
                            perf_mode=DR,
                        )

                    return go

                ob = ob_pool.tile([P, 2048], F32, tag="ob", name="ob")

                def mk_ev(vc):
                    def go():
                        nc.vector.tensor_copy(ob[:, _ts(vc, 512)], pss[vc])

                    return go

                def mk_store():
                    def go():
                        # one 3D-AP DMA stores all 4 v-chunks of the block
                        nc.sync.dma_start(
                            out=out_d[C : C + VDIM, _ts(ib, 512)].rearrange(
                                "(v p) n -> p v n", v=4
                            ),
                            in_=ob.rearrange("p (v n) -> p v n", v=4),
                        )

                    return go

                for m in range(nm - 1):
                    for vc in range(4):
                        pending.append(mk_mm(m, vc))
                # interleave the stop matmuls with evictions so the final
                # block's eviction overlaps its last matmuls
                for vc in range(4):
                    pending.append(mk_mm(nm - 1, vc))
                    pending.append(mk_ev(vc))
                pending.append(mk_store())

            for t in range(4):
                emit_v_tile(t)

            def make_endchain(jc, zp, nslots):
                # strip-end chain: Z, 1/Z, fold into V with +-240 clip
                # (fp8e4 overflows to inf/NaN, no saturation).  Deferred to
                # after the NEXT strip's mask-add so it doesn't delay the
                # exp chain through the in-order DVE queue.
                m, u = jc // 2, jc % 2

                def go():
                    z = zp_pool.tile([P, 1], F32, tag="zf", name="z")
                    nc.vector.reduce_sum(
                        z, zp[:, 0:nslots], axis=mybir.AxisListType.X
                    )
                    nc.vector.reciprocal(zr_s[:, jc : jc + 1], z)
                    vt = vt_pool.tile([P, VDIM], F16, tag="vt", name="vt")
                    nc.vector.tensor_scalar(
                        vt,
                        v16_s[jc],
                        zr_s[:, jc : jc + 1],
                        FP8MAX,
                        op0=mybir.AluOpType.mult,
                        op1=mybir.AluOpType.min,
                    )
                    nc.vector.tensor_scalar_max(
                        v8_s[m][:, _ts(u, VDIM)], vt, -FP8MAX
                    )

                return go

            endchain = None
            for jc in range(NTC):
                i0 = P * jc
                a0 = 512 * (jc // 4)
                m, u = jc // 2, jc % 2
                Lm = T - a0
                if jc + 4 < NTC:
                    emit_v_tile(jc + 4)
                r = jc % 4
                if r > 0:
                    # zero the never-written corner [a0, i0)
                    nc.vector.memset(pt8_s[m][:, u * Lm : u * Lm + P * r], 0.0)
                starts = [i0] + list(range(a0 + 1024, T, 1024))
                nch = len(starts)
                zp = zp_pool.tile([P, 5], F32, tag="zp", name="zp")
                for ci, a in enumerate(starts):
                    b = min(a0 + 1024 * (ci + 1), T)
                    w = b - a
                    ps = s_ps.tile([P, 1024], F32, tag="sps", name="ps_s")
                    segs = [(0, min(w, 512))]
                    if w > 512:
                        segs.append((512, w))
                    for h in range(2):
                        lhs3 = re2(kt8_s[h])[:, :, i0 : i0 + P]
                        for s0, s1 in segs:
                            nc.tensor.matmul(
                                ps[:, s0:s1],
                                lhsT=lhs3,
                                rhs=re2(qt8_s[h])[:, :, a + s0 : a + s1],
                                start=(h == 0),
                                stop=(h == 1),
                                perf_mode=DR,
                            )
                    pt_base = u * Lm + (a - a0)
                    if ci == 0:
                        # split the diagonal chunk: exp of cols [128,w)
                        # doesn't wait for the DVE mask-add on cols [0,128)
                        if w > P:
                            nc.scalar.activation(
                                pt8_s[m][:, pt_base + P : pt_base + w],
                                ps[:, P:w],
                                mybir.ActivationFunctionType.Exp,
                                bias=expshift_s[:, 0:1],
                                scale=1.0,
                                accum_out=zp[:, 0:1],
                            )
                        nc.vector.tensor_add(
                            ps[:, 0:P], ps[:, 0:P], bp_s[:, 8 + VDIM : 8 + VDIM + P]
                        )
                        if endchain is not None:
                            endchain()
                            endchain = None
                        slot_a = nch if w > P else 0
                        nc.scalar.activation(
                            pt8_s[m][:, pt_base : pt_base + P],
                            ps[:, 0:P],
                            mybir.ActivationFunctionType.Exp,
                            bias=expshift_s[:, 0:1],
                            scale=1.0,
                            accum_out=zp[:, slot_a : slot_a + 1],
                        )
                        nslots = nch + 1 if w > P else 1
                    else:
                        nc.scalar.activation(
                            pt8_s[m][:, pt_base : pt_base + w],
                            ps[:, 0:w],
                            mybir.ActivationFunctionType.Exp,
                            bias=expshift_s[:, 0:1],
                            scale=1.0,
                            accum_out=zp[:, ci : ci + 1],
                        )
                    pump(5)
                endchain = make_endchain(jc, zp, nslots)
                if jc % 4 == 3:
                    enqueue_act_block(jc // 4)

            endchain()
            while pending:
                pump(len(pending))

    nc.compile()
    return nc


def _host_inputs(x, Wq, bq, Wk, bk, Wv, bv):
    import ml_dtypes

    def f8(a):
        return np.clip(a, -FP8MAX, FP8MAX).astype(ml_dtypes.float8_e4m3)

    w8 = f8(np.concatenate([Wq.T, Wk.T, Wv.T], axis=1))  # [C, 1536]
    r = np.arange(P)
    mask = np.where(r[None, :] >= r[:, None], 0.0, MASK_NEG).astype(np.float32)
    bp = np.concatenate(
        [
            (bq / C4).reshape(4, P).T,
            (bk / C4).reshape(4, P).T,
            np.tile(bv.astype(np.float32), (P, 1)),
            mask,
        ],
        axis=1,
    ).astype(np.float32)
    bp = np.ascontiguousarray(bp)
    in_maps = []
    for b in range(x.shape[0]):
        xb = np.ascontiguousarray(x[b]).astype(np.float32)
        in_maps.append({"x8": f8(xb), "x32": xb, "w8": w8, "bp": bp})
    return in_maps


def kernel(x, Wq, bq, Wk, bk, Wv, bv, _trace=False, _tmpdir=None):
    import time as _time

    x = np.asarray(x, dtype=np.float32)
    if "nc" not in _CACHE:
        t0 = _time.time()
        _CACHE["nc"] = build_nc()
        print(f"[kernel] build_nc done in {_time.time() - t0:.1f}s", flush=True)
    nc = _CACHE["nc"]
    in_maps = _host_inputs(
        x,
        np.asarray(Wq, np.float32),
        np.asarray(bq, np.float32),
        np.asarray(Wk, np.float32),
        np.asarray(bk, np.float32),
        np.asarray(Wv, np.float32),
        np.asarray(bv, np.float32),
    )
    t0 = _time.time()
    res = run_bass_kernel_spmd(
        nc, in_maps, core_ids=list(range(8)), trace=_trace, tmpdir=_tmpdir
    )
    print(f"[kernel] run done in {_time.time() - t0:.1f}s", flush=True)
    _CACHE["last_result"] = res
    out = np.stack([r["out"] for r in res.results]).astype(np.float32)
    return out


# revision 27
# speedup vs baseline: 3.2988x; 1.1040x over previous
"""Trainium2 Bass kernel for nn_AttentionBlock (sparse_attention).

Reference computation per batch b (channels-first x[b]: [C=512, T=4096]):
    xt = x[b].T                                  # [T, C]
    q = xt @ Wq.T + bq ; k = xt @ Wk.T + bk      # [T, 512]
    v = xt @ Wv.T + bv                           # [T, 512]
    S = q @ k.T / sqrt(512), causal (j <= i)     # [T, T]
    P = softmax(S, axis=QUERY i)  (per-column normalization)
    act = P @ v                                  # [T, 512]
    out[b] = concat(x[b], act.T, axis=0)         # [1024, T]

Sharding: pure data-parallel over batch B=8 across the 8 NeuronCores
(one batch per core, no collectives).

v3 per-core algorithm — all matmuls fp8e4 DoubleRow (f32 PSUM):
  1. QKV projections from x8 (host-cast fp8).  Contraction over C=512
     as 2 DoubleRow pairs.  Q^T,K^T stored fp8 paired over head-dim
     chunks; V rows fp16.  Q/K eviction on DVE: (psum/c4) + b/c4 ->
     fp8 (c4 = 512**0.25 splits the score scale between q and k).
     Weights+biases+mask ride in 2 packed DRAM params (DMA triggers
     cost ~650ns each on the Sync engine; fewer, bigger transfers).
  2. Score strips ST[j,i] = K^T.T @ Q^T (j-chunk of 128 keys, i from
     the diagonal to T in 1024-col PSUM chunks; 512-col DoubleRow MMs,
     h-outer so LDWEIGHTS amortizes 2 MMs).  Column softmax over i:
     additive causal mask on the diagonal block, exp(s-4) on ScalarE
     with accum_out producing Z_j partials (1024-col chunks halve the
     per-instruction ACTIVATE/READ_ACCUMULATOR overhead).  P~ stays in
     SBUF as fp8 pair tiles (triangle = 72KB/partition).
  3. V rows scaled by 1/Z_j on DVE, clipped to +-240 (fp8e4 overflows
     to inf/NaN, no saturation), cast fp8 into paired v8 tiles.
  4. act^T[v,i] = sum_j V'[j,v] * P~[j,i]: PSUM-accumulated DoubleRow
     matmuls from SBUF, interleaved between score-strip chunks so
     TensorE never stalls on the ScalarE exp chain.  DVE eviction.
  5. out rows 0..511 are a DRAM->DRAM copy of x[b] overlapping the
     whole kernel.
"""

import math

import numpy as np

import concourse.bass as bass
import concourse.mybir as mybir
from concourse import bacc, tile
from concourse.bass_utils import run_bass_kernel_spmd

P = 128
C = 512
T = 4096
KDIM = 512
VDIM = 512
NTC = T // P      # 32 time chunks of 128
NIB = T // 512    # 8 i-blocks of 512
F16 = mybir.dt.float16
F32 = mybir.dt.float32
F8 = mybir.dt.float8e4
EXP_SHIFT = -4.0  # constant logit shift: softmax-invariant, keeps exp in range
MASK_NEG = -10000.0
C4 = float(C) ** 0.25
FP8MAX = 240.0
DR = mybir.MatmulPerfMode.DoubleRow
WPK = 3 * KDIM    # packed weight row: wq | wk | wv
BPK = 4 + 4 + VDIM + P  # packed per-partition consts: bq | bk | bv | mask

_CACHE = {}


def _ts(i, size):
    return slice(i * size, (i + 1) * size)


def build_nc():
    nc = bacc.Bacc(
        "TRN2",
        target_bir_lowering=False,
        debug=False,
        num_devices=8,
    )

    x8_d = nc.declare_dram_parameter("x8", [C, T], F8, isOutput=False)
    x32_d = nc.declare_dram_parameter("x32", [C, T], F32, isOutput=False)
    w8_d = nc.declare_dram_parameter("w8", [C, WPK], F8, isOutput=False)
    bp_d = nc.declare_dram_parameter("bp", [P, BPK], F32, isOutput=False)
    out_d = nc.declare_dram_parameter("out", [C + VDIM, T], F32, isOutput=True)

    def re2(ap):
        return ap.rearrange("p (u n) -> p u n", u=2)

    with tile.TileContext(nc) as tc:
        from contextlib import ExitStack

        with ExitStack() as ctx:
            singles = ctx.enter_context(tc.tile_pool(name="singles", bufs=1))

            def single(shape, dtype, tag):
                return singles.tile(shape, dtype, name=tag, tag=tag)

            # paired fp8 layouts: plane u of tile h holds 128-chunk 2h+u
            x8_s = [single([P, 2 * T], F8, f"x8s{h}") for h in range(2)]
            w8_s = [single([P, 2 * WPK], F8, f"w8s{h}") for h in range(2)]
            bp_s = single([P, BPK], F32, "bps")
            # packed layout: bq[0:4] | bk[4:8] | bv[8:520] | mask[520:648]
            qt8_s = [single([P, 2 * T], F8, f"qt8s{h}") for h in range(2)]
            kt8_s = [single([P, 2 * T], F8, f"kt8s{h}") for h in range(2)]
            v16_s = [single([P, VDIM], F16, f"v16s{t}") for t in range(NTC)]
            v8_s = [single([P, 2 * VDIM], F8, f"v8s{m}") for m in range(NTC // 2)]
            # P~ fp8 pair tiles: pair m holds strips jc=2m,2m+1; valid
            # i >= a0 = 512*(m//2); plane length Lm = T - a0
            pt8_s = []
            for m in range(NTC // 2):
                Lm = T - 512 * (m // 2)
                pt8_s.append(single([P, 2 * Lm], F8, f"pt8s{m}"))
            zr_s = single([P, NTC], F32, "zrs")
            expshift_s = single([P, 1], F32, "expshift")
            nc.vector.memset(expshift_s, EXP_SHIFT)

            # ---- input DMAs: triggers cost ~610ns each and serialize on
            # the Sync queue, so batch via 3D APs (both u-planes per DMA)
            # and put compute-critical transfers first ----
            for h in range(2):
                nc.sync.dma_start(
                    out=re2(w8_s[h]),
                    in_=w8_d[2 * h * P : (2 * h + 2) * P, :].rearrange(
                        "(u p) n -> p u n", u=2
                    ),
                )
            # x8 in two column pieces so the first QKV group unblocks early
            for piece in range(2):
                for h in range(2):
                    nc.sync.dma_start(
                        out=re2(x8_s[h])[:, :, _ts(piece, 2048)],
                        in_=x8_d[
                            2 * h * P : (2 * h + 2) * P, _ts(piece, 2048)
                        ].rearrange("(u p) n -> p u n", u=2),
                    )
                if piece == 0:
                    nc.sync.dma_start(out=bp_s, in_=bp_d[:, :])
            # x copy-through rows 0..511 (DRAM->DRAM), overlaps everything
            nc.sync.dma_start(out=out_d[0:C, :], in_=x32_d[:, :])

            # ---- Phase QKV: Q, K projections (fp8 DoubleRow) ----
            qkv_ps_cm = tc.tile_pool(name="qkv_ps", bufs=8, space="PSUM")
            qkv_ps = qkv_ps_cm.__enter__()
            for wofs, bofs, dst in ((0, 0, qt8_s), (KDIM, 4, kt8_s)):
                for g in range(2):
                    for kk in range(4):
                        pss = [
                            qkv_ps.tile([P, 512], F32, tag="qkvps", name="ps_qk")
                            for _ in range(4)
                        ]
                        for h in range(2):
                            lhs3 = re2(w8_s[h])[:, :, wofs + kk * P : wofs + (kk + 1) * P]
                            for gi in range(4):
                                ib = 4 * g + gi
                                nc.tensor.matmul(
                                    pss[gi],
                                    lhsT=lhs3,
                                    rhs=re2(x8_s[h])[:, :, _ts(ib, 512)],
                                    start=(h == 0),
                                    stop=(h == 1),
                                    perf_mode=DR,
                                )
                        for gi in range(4):
                            ib = 4 * g + gi
                            dst_ap = dst[kk // 2][
                                :, (kk % 2) * T + ib * 512 : (kk % 2) * T + ib * 512 + 512
                            ]
                            # evictions pace QKV: split across DVE/ScalarE
                            if gi % 2 == 0:
                                nc.vector.tensor_scalar(
                                    dst_ap,
                                    pss[gi],
                                    1.0 / C4,
                                    bp_s[:, bofs + kk : bofs + kk + 1],
                                    op0=mybir.AluOpType.mult,
                                    op1=mybir.AluOpType.add,
                                )
                            else:
                                nc.scalar.activation(
                                    dst_ap,
                                    pss[gi],
                                    mybir.ActivationFunctionType.Identity,
                                    bias=bp_s[:, bofs + kk : bofs + kk + 1],
                                    scale=1.0 / C4,
                                )
            qkv_ps_cm.__exit__(None, None, None)

            # ---- Phase 1 + 2 interleaved ----
            # s_ps: [128,1024] f32 tiles (2 PSUM banks), bufs=3 -> 6 banks
            # act_ps: 2 x [128,512] (1 bank each) -> 2 banks
            s_ps = ctx.enter_context(tc.tile_pool(name="s_ps", bufs=3, space="PSUM"))
            act_ps = ctx.enter_context(
                tc.tile_pool(name="act_ps", bufs=1, space="PSUM")
            )
            zp_pool = ctx.enter_context(tc.tile_pool(name="zp", bufs=4))
            vt_pool = ctx.enter_context(tc.tile_pool(name="vt", bufs=4))
            ob_pool = ctx.enter_context(tc.tile_pool(name="ob", bufs=2))

            # work queue of deferred act-block ops (closures), pumped
            # between strip chunks so TensorE never idles on the exp chain
            pending = []

            def pump(n):
                for _ in range(min(n, len(pending))):
                    pending.pop(0)()

            def emit_v_tile(t):
                # V projection tile t; borrows an s_ps slot (uses half)
                ps = s_ps.tile([P, 1024], F32, tag="sps", name="ps_v")
                for h in range(2):
                    nc.tensor.matmul(
                        ps[:, 0:512],
                        lhsT=re2(x8_s[h])[:, :, _ts(t, P)],
                        rhs=re2(w8_s[h])[:, :, 2 * KDIM : 3 * KDIM],
                        start=(h == 0),
                        stop=(h == 1),
                        perf_mode=DR,
                    )
                nc.vector.tensor_add(v16_s[t], ps[:, 0:512], bp_s[:, 8 : 8 + VDIM])

            def enqueue_act_block(ib):
                nm = 2 * (ib + 1)  # pairs m contributing to block ib
                ob = ob_pool.tile([P, 2048], F32, tag="ob", name="ob")

                def mk_mm(ps, m, vc):
                    def go():
                        off = 512 * ib - 512 * (m // 2)
                        nc.tensor.matmul(
                            ps,
                            lhsT=re2(v8_s[m])[:, :, _ts(vc, P)],
                            rhs=re2(pt8_s[m])[:, :, off : off + 512],
                            start=(m == 0),
                            stop=(m == nm - 1),
                            perf_mode=DR,
                        )

                    return go

                def mk_ev(ps, vc):
                    def go():
                        nc.vector.tensor_copy(ob[:, _ts(vc, 512)], ps)

                    return go

                def mk_store():
                    def go():
                        # one 3D-AP DMA stores all 4 v-chunks of the block
                        nc.sync.dma_start(
                            out=out_d[C : C + VDIM, _ts(ib, 512)].rearrange(
                                "(v p) n -> p v n", v=4
                            ),
                            in_=ob.rearrange("p (v n) -> p v n", v=4),
                        )

                    return go

                # two vc-half passes over 2 PSUM tiles (frees banks for a
                # deeper s_ps ring); stop matmuls interleave with evictions
                for half in range(2):
                    pss = [
                        act_ps.tile([P, 512], F32, tag=f"aps{v}", name=f"aps{v}")
                        for v in range(2)
                    ]
                    for m in range(nm - 1):
                        for vi in range(2):
                            pending.append(mk_mm(pss[vi], m, 2 * half + vi))
                    for vi in range(2):
                        pending.append(mk_mm(pss[vi], nm - 1, 2 * half + vi))
                        pending.append(mk_ev(pss[vi], 2 * half + vi))
                pending.append(mk_store())

            for t in range(4):
                emit_v_tile(t)

            def make_endchain(jc, zp, nslots):
                # strip-end chain: Z, 1/Z, fold into V with +-240 clip
                # (fp8e4 overflows to inf/NaN, no saturation).  Deferred to
                # after the NEXT strip's mask-add so it doesn't delay the
                # exp chain through the in-order DVE queue.
                m, u = jc // 2, jc % 2

                def go():
                    z = zp_pool.tile([P, 1], F32, tag="zf", name="z")
                    nc.vector.reduce_sum(
                        z, zp[:, 0:nslots], axis=mybir.AxisListType.X
                    )
                    nc.vector.reciprocal(zr_s[:, jc : jc + 1], z)
                    vt = vt_pool.tile([P, VDIM], F16, tag="vt", name="vt")
                    nc.vector.tensor_scalar(
                        vt,
                        v16_s[jc],
                        zr_s[:, jc : jc + 1],
                        FP8MAX,
                        op0=mybir.AluOpType.mult,
                        op1=mybir.AluOpType.min,
                    )
                    nc.vector.tensor_scalar_max(
                        v8_s[m][:, _ts(u, VDIM)], vt, -FP8MAX
                    )

                return go

            endchain = None
            for jc in range(NTC):
                i0 = P * jc
                a0 = 512 * (jc // 4)
                m, u = jc // 2, jc % 2
                Lm = T - a0
                if jc + 4 < NTC:
                    emit_v_tile(jc + 4)
                r = jc % 4
                if r > 0:
                    # zero the never-written corner [a0, i0)
                    nc.vector.memset(pt8_s[m][:, u * Lm : u * Lm + P * r], 0.0)
                starts = [i0] + list(range(a0 + 1024, T, 1024))
                nch = len(starts)
                zp = zp_pool.tile([P, 5], F32, tag="zp", name="zp")
                for ci, a in enumerate(starts):
                    b = min(a0 + 1024 * (ci + 1), T)
                    w = b - a
                    ps = s_ps.tile([P, 1024], F32, tag="sps", name="ps_s")
                    segs = [(0, min(w, 512))]
                    if w > 512:
                        segs.append((512, w))
                    for h in range(2):
                        lhs3 = re2(kt8_s[h])[:, :, i0 : i0 + P]
                        for s0, s1 in segs:
                            nc.tensor.matmul(
                                ps[:, s0:s1],
                                lhsT=lhs3,
                                rhs=re2(qt8_s[h])[:, :, a + s0 : a + s1],
                                start=(h == 0),
                                stop=(h == 1),
                                perf_mode=DR,
                            )
                    pt_base = u * Lm + (a - a0)
                    if ci == 0:
                        # split the diagonal chunk: exp of cols [128,w)
                        # doesn't wait for the DVE mask-add on cols [0,128)
                        if w > P:
                            nc.scalar.activation(
                                pt8_s[m][:, pt_base + P : pt_base + w],
                                ps[:, P:w],
                                mybir.ActivationFunctionType.Exp,
                                bias=expshift_s[:, 0:1],
                                scale=1.0,
                                accum_out=zp[:, 0:1],
                            )
                        nc.vector.tensor_add(
                            ps[:, 0:P], ps[:, 0:P], bp_s[:, 8 + VDIM : 8 + VDIM + P]
                        )
                        if endchain is not None:
                            endchain()
                            endchain = None
                        slot_a = nch if w > P else 0
                        nc.scalar.activation(
                            pt8_s[m][:, pt_base : pt_base + P],
                            ps[:, 0:P],
                            mybir.ActivationFunctionType.Exp,
                            bias=expshift_s[:, 0:1],
                            scale=1.0,
                            accum_out=zp[:, slot_a : slot_a + 1],
                        )
                        nslots = nch + 1 if w > P else 1
                    else:
                        nc.scalar.activation(
                            pt8_s[m][:, pt_base : pt_base + w],
                            ps[:, 0:w],
                            mybir.ActivationFunctionType.Exp,
                            bias=expshift_s[:, 0:1],
                            scale=1.0,
                            accum_out=zp[:, ci : ci + 1],
                        )
                    pump(5)
                endchain = make_endchain(jc, zp, nslots)
                if jc % 4 == 3:
                    enqueue_act_block(jc // 4)

            endchain()
            while pending:
                pump(len(pending))

    nc.compile()
    return nc


def _host_inputs(x, Wq, bq, Wk, bk, Wv, bv):
    import ml_dtypes

    def f8(a):
        return np.clip(a, -FP8MAX, FP8MAX).astype(ml_dtypes.float8_e4m3)

    w8 = f8(np.concatenate([Wq.T, Wk.T, Wv.T], axis=1))  # [C, 1536]
    r = np.arange(P)
    mask = np.where(r[None, :] >= r[:, None], 0.0, MASK_NEG).astype(np.float32)
    bp = np.concatenate(
        [
            (bq / C4).reshape(4, P).T,
            (bk / C4).reshape(4, P).T,
            np.tile(bv.astype(np.float32), (P, 1)),
            mask,
        ],
        axis=1,
    ).astype(np.float32)
    bp = np.ascontiguousarray(bp)
    in_maps = []
    for b in range(x.shape[0]):
        xb = np.ascontiguousarray(x[b]).astype(np.float32)
        in_maps.append({"x8": f8(xb), "x32": xb, "w8": w8, "bp": bp})
    return in_maps


def kernel(x, Wq, bq, Wk, bk, Wv, bv, _trace=False, _tmpdir=None):
    import time as _time

    x = np.asarray(x, dtype=np.float32)
    if "nc" not in _CACHE:
        t0 = _time.time()
        _CACHE["nc"] = build_nc()
        print(f"[kernel] build_nc done in {_time.time() - t0:.1f}s", flush=True)
    nc = _CACHE["nc"]
    in_maps = _host_inputs(
        x,
        np.asarray(Wq, np.float32),
        np.asarray(bq, np.float32),
        np.asarray(Wk, np.float32),
        np.asarray(bk, np.float32),
        np.asarray(Wv, np.float32),
        np.asarray(bv, np.float32),
    )
    t0 = _time.time()
    res = run_bass_kernel_spmd(
        nc, in_maps, core_ids=list(range(8)), trace=_trace, tmpdir=_tmpdir
    )
    print(f"[kernel] run done in {_time.time() - t0:.1f}s", flush=True)
    _CACHE["last_result"] = res
    out = np.stack([r["out"] for r in res.results]).astype(np.float32)
    return out


# revision 32
# speedup vs baseline: 3.3479x; 1.0149x over previous
"""Trainium2 Bass kernel for nn_AttentionBlock (sparse_attention).

Reference computation per batch b (channels-first x[b]: [C=512, T=4096]):
    xt = x[b].T                                  # [T, C]
    q = xt @ Wq.T + bq ; k = xt @ Wk.T + bk      # [T, 512]
    v = xt @ Wv.T + bv                           # [T, 512]
    S = q @ k.T / sqrt(512), causal (j <= i)     # [T, T]
    P = softmax(S, axis=QUERY i)  (per-column normalization)
    act = P @ v                                  # [T, 512]
    out[b] = concat(x[b], act.T, axis=0)         # [1024, T]

Sharding: pure data-parallel over batch B=8 across the 8 NeuronCores
(one batch per core, no collectives).

v3 per-core algorithm — all matmuls fp8e4 DoubleRow (f32 PSUM):
  1. QKV projections from x8 (host-cast fp8).  Contraction over C=512
     as 2 DoubleRow pairs.  Q^T,K^T stored fp8 paired over head-dim
     chunks; V rows fp16.  Q/K eviction on DVE: (psum/c4) + b/c4 ->
     fp8 (c4 = 512**0.25 splits the score scale between q and k).
     Weights+biases+mask ride in 2 packed DRAM params (DMA triggers
     cost ~650ns each on the Sync engine; fewer, bigger transfers).
  2. Score strips ST[j,i] = K^T.T @ Q^T (j-chunk of 128 keys, i from
     the diagonal to T in 1024-col PSUM chunks; 512-col DoubleRow MMs,
     h-outer so LDWEIGHTS amortizes 2 MMs).  Column softmax over i:
     additive causal mask on the diagonal block, exp(s-4) on ScalarE
     with accum_out producing Z_j partials (1024-col chunks halve the
     per-instruction ACTIVATE/READ_ACCUMULATOR overhead).  P~ stays in
     SBUF as fp8 pair tiles (triangle = 72KB/partition).
  3. V rows scaled by 1/Z_j on DVE, clipped to +-240 (fp8e4 overflows
     to inf/NaN, no saturation), cast fp8 into paired v8 tiles.
  4. act^T[v,i] = sum_j V'[j,v] * P~[j,i]: PSUM-accumulated DoubleRow
     matmuls from SBUF, interleaved between score-strip chunks so
     TensorE never stalls on the ScalarE exp chain.  DVE eviction.
  5. out rows 0..511 are a DRAM->DRAM copy of x[b] overlapping the
     whole kernel.
"""

import math

import numpy as np

import concourse.bass as bass
import concourse.mybir as mybir
from concourse import bacc, tile
from concourse.bass_utils import run_bass_kernel_spmd

P = 128
C = 512
T = 4096
KDIM = 512
VDIM = 512
NTC = T // P      # 32 time chunks of 128
NIB = T // 512    # 8 i-blocks of 512
F16 = mybir.dt.float16
F32 = mybir.dt.float32
F8 = mybir.dt.float8e4
EXP_SHIFT = -4.0  # constant logit shift: softmax-invariant, keeps exp in range
MASK_NEG = -10000.0
C4 = float(C) ** 0.25
FP8MAX = 240.0
DR = mybir.MatmulPerfMode.DoubleRow
WPK = 3 * KDIM    # packed weight row: wq | wk | wv
BPK = 4 + 4 + VDIM + P  # packed per-partition consts: bq | bk | bv | mask

_CACHE = {}


def _ts(i, size):
    return slice(i * size, (i + 1) * size)


def build_nc():
    nc = bacc.Bacc(
        "TRN2",
        target_bir_lowering=False,
        debug=False,
        num_devices=8,
    )

    x8_d = nc.declare_dram_parameter("x8", [C, T], F8, isOutput=False)
    x32_d = nc.declare_dram_parameter("x32", [C, T], F32, isOutput=False)
    w8_d = nc.declare_dram_parameter("w8", [C, WPK], F8, isOutput=False)
    bp_d = nc.declare_dram_parameter("bp", [P, BPK], F32, isOutput=False)
    out_d = nc.declare_dram_parameter("out", [C + VDIM, T], F32, isOutput=True)

    def re2(ap):
        return ap.rearrange("p (u n) -> p u n", u=2)

    with tile.TileContext(nc) as tc:
        from contextlib import ExitStack

        with ExitStack() as ctx:
            singles = ctx.enter_context(tc.tile_pool(name="singles", bufs=1))

            def single(shape, dtype, tag):
                return singles.tile(shape, dtype, name=tag, tag=tag)

            # paired fp8 layouts: plane u of tile h holds 128-chunk 2h+u
            x8_s = [single([P, 2 * T], F8, f"x8s{h}") for h in range(2)]
            w8_s = [single([P, 2 * WPK], F8, f"w8s{h}") for h in range(2)]
            bp_s = single([P, BPK], F32, "bps")
            # packed layout: bq[0:4] | bk[4:8] | bv[8:520] | mask[520:648]
            qt8_s = [single([P, 2 * T], F8, f"qt8s{h}") for h in range(2)]
            kt8_s = [single([P, 2 * T], F8, f"kt8s{h}") for h in range(2)]
            v16_s = [single([P, VDIM], F16, f"v16s{t}") for t in range(NTC)]
            v8_s = [single([P, 2 * VDIM], F8, f"v8s{m}") for m in range(NTC // 2)]
            # P~ fp8 pair tiles: pair m holds strips jc=2m,2m+1; valid
            # i >= a0 = 512*(m//2); plane length Lm = T - a0
            pt8_s = []
            for m in range(NTC // 2):
                Lm = T - 512 * (m // 2)
                pt8_s.append(single([P, 2 * Lm], F8, f"pt8s{m}"))
            zr_s = single([P, NTC], F32, "zrs")
            expshift_s = single([P, 1], F32, "expshift")
            nc.vector.memset(expshift_s, EXP_SHIFT)

            # ---- input DMAs: triggers cost ~610ns each and serialize on
            # the Sync queue, so batch via 3D APs (both u-planes per DMA)
            # and put compute-critical transfers first ----
            for h in range(2):
                nc.sync.dma_start(
                    out=re2(w8_s[h]),
                    in_=w8_d[2 * h * P : (2 * h + 2) * P, :].rearrange(
                        "(u p) n -> p u n", u=2
                    ),
                )
            # x8 in two column pieces so the first QKV group unblocks early
            for piece in range(2):
                for h in range(2):
                    nc.sync.dma_start(
                        out=re2(x8_s[h])[:, :, _ts(piece, 2048)],
                        in_=x8_d[
                            2 * h * P : (2 * h + 2) * P, _ts(piece, 2048)
                        ].rearrange("(u p) n -> p u n", u=2),
                    )
                if piece == 0:
                    nc.sync.dma_start(out=bp_s, in_=bp_d[:, :])
            # x copy-through rows 0..511 (DRAM->DRAM), overlaps everything
            nc.sync.dma_start(out=out_d[0:C, :], in_=x32_d[:, :])

            # ---- Phase QKV: Q, K projections (fp8 DoubleRow) ----
            qkv_ps_cm = tc.tile_pool(name="qkv_ps", bufs=8, space="PSUM")
            qkv_ps = qkv_ps_cm.__enter__()
            for wofs, bofs, dst in ((0, 0, qt8_s), (KDIM, 4, kt8_s)):
                for g in range(2):
                    for kk in range(4):
                        pss = [
                            qkv_ps.tile([P, 512], F32, tag="qkvps", name="ps_qk")
                            for _ in range(4)
                        ]
                        for h in range(2):
                            lhs3 = re2(w8_s[h])[:, :, wofs + kk * P : wofs + (kk + 1) * P]
                            for gi in range(4):
                                ib = 4 * g + gi
                                nc.tensor.matmul(
                                    pss[gi],
                                    lhsT=lhs3,
                                    rhs=re2(x8_s[h])[:, :, _ts(ib, 512)],
                                    start=(h == 0),
                                    stop=(h == 1),
                                    perf_mode=DR,
                                )
                        for gi in range(4):
                            ib = 4 * g + gi
                            dst_ap = dst[kk // 2][
                                :, (kk % 2) * T + ib * 512 : (kk % 2) * T + ib * 512 + 512
                            ]
                            # evictions pace QKV: split across DVE/ScalarE
                            if gi % 2 == 0:
                                nc.vector.tensor_scalar(
                                    dst_ap,
                                    pss[gi],
                                    1.0 / C4,
                                    bp_s[:, bofs + kk : bofs + kk + 1],
                                    op0=mybir.AluOpType.mult,
                                    op1=mybir.AluOpType.add,
                                )
                            else:
                                nc.scalar.activation(
                                    dst_ap,
                                    pss[gi],
                                    mybir.ActivationFunctionType.Identity,
                                    bias=bp_s[:, bofs + kk : bofs + kk + 1],
                                    scale=1.0 / C4,
                                )
            qkv_ps_cm.__exit__(None, None, None)

            # ---- Phase 1 + 2 interleaved ----
            # s_ps: [128,1024] f32 tiles (2 PSUM banks), bufs=3 -> 6 banks
            # act_ps: 2 x [128,512] (1 bank each) -> 2 banks
            s_ps = ctx.enter_context(tc.tile_pool(name="s_ps", bufs=3, space="PSUM"))
            act_ps = ctx.enter_context(
                tc.tile_pool(name="act_ps", bufs=1, space="PSUM")
            )
            zp_pool = ctx.enter_context(tc.tile_pool(name="zp", bufs=4))
            vt_pool = ctx.enter_context(tc.tile_pool(name="vt", bufs=4))
            ob_pool = ctx.enter_context(tc.tile_pool(name="ob", bufs=2))

            # work queue of deferred act-block ops (closures), pumped
            # between strip chunks so TensorE never idles on the exp chain
            pending = []

            def pump(n):
                for _ in range(min(n, len(pending))):
                    pending.pop(0)()

            def emit_v_tile(t):
                # V projection tile t; borrows an s_ps slot (uses half)
                ps = s_ps.tile([P, 1024], F32, tag="sps", name="ps_v")
                for h in range(2):
                    nc.tensor.matmul(
                        ps[:, 0:512],
                        lhsT=re2(x8_s[h])[:, :, _ts(t, P)],
                        rhs=re2(w8_s[h])[:, :, 2 * KDIM : 3 * KDIM],
                        start=(h == 0),
                        stop=(h == 1),
                        perf_mode=DR,
                    )
                nc.vector.tensor_add(v16_s[t], ps[:, 0:512], bp_s[:, 8 : 8 + VDIM])

            def enqueue_act_block(ib):
                nm = 2 * (ib + 1)  # pairs m contributing to block ib
                ob = ob_pool.tile([P, 2048], F32, tag="ob", name="ob")

                def mk_mm(ps, m, vc):
                    def go():
                        off = 512 * ib - 512 * (m // 2)
                        nc.tensor.matmul(
                            ps,
                            lhsT=re2(v8_s[m])[:, :, _ts(vc, P)],
                            rhs=re2(pt8_s[m])[:, :, off : off + 512],
                            start=(m == 0),
                            stop=(m == nm - 1),
                            perf_mode=DR,
                        )

                    return go

                def mk_ev(ps, vc):
                    def go():
                        nc.vector.tensor_copy(ob[:, _ts(vc, 512)], ps)

                    return go

                def mk_store(half):
                    def go():
                        # per-half 3D-AP DMA: half 0 transfers while the
                        # second half's PSUM is still evicting
                        nc.sync.dma_start(
                            out=out_d[
                                C + half * 2 * P : C + (half + 1) * 2 * P,
                                _ts(ib, 512),
                            ].rearrange("(v p) n -> p v n", v=2),
                            in_=ob[:, half * 1024 : (half + 1) * 1024].rearrange(
                                "p (v n) -> p v n", v=2
                            ),
                        )

                    return go

                # two vc-half passes over 2 PSUM tiles (frees banks for a
                # deeper s_ps ring); stop matmuls interleave with evictions
                for half in range(2):
                    pss = [
                        act_ps.tile([P, 512], F32, tag=f"aps{v}", name=f"aps{v}")
                        for v in range(2)
                    ]
                    for m in range(nm - 1):
                        for vi in range(2):
                            pending.append(mk_mm(pss[vi], m, 2 * half + vi))
                    for vi in range(2):
                        pending.append(mk_mm(pss[vi], nm - 1, 2 * half + vi))
                        pending.append(mk_ev(pss[vi], 2 * half + vi))
                    pending.append(mk_store(half))

            for t in range(4):
                emit_v_tile(t)

            def make_endchain(jc, zp, nslots):
                # strip-end chain: Z, 1/Z, fold into V with +-240 clip
                # (fp8e4 overflows to inf/NaN, no saturation).  Deferred to
                # after the NEXT strip's mask-add so it doesn't delay the
                # exp chain through the in-order DVE queue.
                m, u = jc // 2, jc % 2

                def go():
                    z = zp_pool.tile([P, 1], F32, tag="zf", name="z")
                    nc.vector.reduce_sum(
                        z, zp[:, 0:nslots], axis=mybir.AxisListType.X
                    )
                    nc.vector.reciprocal(zr_s[:, jc : jc + 1], z)
                    vt = vt_pool.tile([P, VDIM], F16, tag="vt", name="vt")
                    nc.vector.tensor_scalar(
                        vt,
                        v16_s[jc],
                        zr_s[:, jc : jc + 1],
                        FP8MAX,
                        op0=mybir.AluOpType.mult,
                        op1=mybir.AluOpType.min,
                    )
                    nc.vector.tensor_scalar_max(
                        v8_s[m][:, _ts(u, VDIM)], vt, -FP8MAX
                    )

                return go

            endchain = None
            for jc in range(NTC):
                i0 = P * jc
                a0 = 512 * (jc // 4)
                m, u = jc // 2, jc % 2
                Lm = T - a0
                if jc + 4 < NTC:
                    emit_v_tile(jc + 4)
                r = jc % 4
                if r > 0:
                    # zero the never-written corner [a0, i0)
                    nc.vector.memset(pt8_s[m][:, u * Lm : u * Lm + P * r], 0.0)
                starts = [i0] + list(range(a0 + 1024, T, 1024))
                nch = len(starts)
                zp = zp_pool.tile([P, 5], F32, tag="zp", name="zp")
                for ci, a in enumerate(starts):
                    b = min(a0 + 1024 * (ci + 1), T)
                    w = b - a
                    ps = s_ps.tile([P, 1024], F32, tag="sps", name="ps_s")
                    segs = [(0, min(w, 512))]
                    if w > 512:
                        segs.append((512, w))
                    for h in range(2):
                        lhs3 = re2(kt8_s[h])[:, :, i0 : i0 + P]
                        for s0, s1 in segs:
                            nc.tensor.matmul(
                                ps[:, s0:s1],
                                lhsT=lhs3,
                                rhs=re2(qt8_s[h])[:, :, a + s0 : a + s1],
                                start=(h == 0),
                                stop=(h == 1),
                                perf_mode=DR,
                            )
                    pt_base = u * Lm + (a - a0)
                    if ci == 0:
                        # split the diagonal chunk: exp of cols [128,w)
                        # doesn't wait for the DVE mask-add on cols [0,128)
                        if w > P:
                            nc.scalar.activation(
                                pt8_s[m][:, pt_base + P : pt_base + w],
                                ps[:, P:w],
                                mybir.ActivationFunctionType.Exp,
                                bias=expshift_s[:, 0:1],
                                scale=1.0,
                                accum_out=zp[:, 0:1],
                            )
                        nc.vector.tensor_add(
                            ps[:, 0:P], ps[:, 0:P], bp_s[:, 8 + VDIM : 8 + VDIM + P]
                        )
                        if endchain is not None:
                            endchain()
                            endchain = None
                        slot_a = nch if w > P else 0
                        nc.scalar.activation(
                            pt8_s[m][:, pt_base : pt_base + P],
                            ps[:, 0:P],
                            mybir.ActivationFunctionType.Exp,
                            bias=expshift_s[:, 0:1],
                            scale=1.0,
                            accum_out=zp[:, slot_a : slot_a + 1],
                        )
                        nslots = nch + 1 if w > P else 1
                    else:
                        nc.scalar.activation(
                            pt8_s[m][:, pt_base : pt_base + w],
                            ps[:, 0:w],
                            mybir.ActivationFunctionType.Exp,
                            bias=expshift_s[:, 0:1],
                            scale=1.0,
                            accum_out=zp[:, ci : ci + 1],
                        )
                    pump(5)
                endchain = make_endchain(jc, zp, nslots)
                if jc % 4 == 3:
                    enqueue_act_block(jc // 4)

            endchain()
            while pending:
                pump(len(pending))

    nc.compile()
    return nc


def _host_inputs(x, Wq, bq, Wk, bk, Wv, bv):
    import ml_dtypes

    def f8(a):
        return np.clip(a, -FP8MAX, FP8MAX).astype(ml_dtypes.float8_e4m3)

    w8 = f8(np.concatenate([Wq.T, Wk.T, Wv.T], axis=1))  # [C, 1536]
    r = np.arange(P)
    mask = np.where(r[None, :] >= r[:, None], 0.0, MASK_NEG).astype(np.float32)
    bp = np.concatenate(
        [
            (bq / C4).reshape(4, P).T,
            (bk / C4).reshape(4, P).T,
            np.tile(bv.astype(np.float32), (P, 1)),
            mask,
        ],
        axis=1,
    ).astype(np.float32)
    bp = np.ascontiguousarray(bp)
    in_maps = []
    for b in range(x.shape[0]):
        xb = np.ascontiguousarray(x[b]).astype(np.float32)
        in_maps.append({"x8": f8(xb), "x32": xb, "w8": w8, "bp": bp})
    return in_maps


def kernel(x, Wq, bq, Wk, bk, Wv, bv, _trace=False, _tmpdir=None):
    import time as _time

    x = np.asarray(x, dtype=np.float32)
    if "nc" not in _CACHE:
        t0 = _time.time()
        _CACHE["nc"] = build_nc()
        print(f"[kernel] build_nc done in {_time.time() - t0:.1f}s", flush=True)
    nc = _CACHE["nc"]
    in_maps = _host_inputs(
        x,
        np.asarray(Wq, np.float32),
        np.asarray(bq, np.float32),
        np.asarray(Wk, np.float32),
        np.asarray(bk, np.float32),
        np.asarray(Wv, np.float32),
        np.asarray(bv, np.float32),
    )
    t0 = _time.time()
    res = run_bass_kernel_spmd(
        nc, in_maps, core_ids=list(range(8)), trace=_trace, tmpdir=_tmpdir
    )
    print(f"[kernel] run done in {_time.time() - t0:.1f}s", flush=True)
    _CACHE["last_result"] = res
    out = np.stack([r["out"] for r in res.results]).astype(np.float32)
    return out
